# revision 23
# baseline (speedup 1.0000x reference)
"""Distributed multi-head attention on 8 Trainium2 NeuronCores (Bass/Tile).

Problem: x[4,2048,1024] f32; q = x@Wq, kv = x@Wkv, 16 heads x 64;
softmax(q k^T / sqrt(64)) @ v; out @ Wo + bo.

Sharding (no collectives): 8 cores = 4 batches x 2 head-groups.
Each core computes 8 heads (inner dims g*512..g*512+512) over the full
sequence for one batch:
  Q/K/V projections use only its 512 columns of Wq/Wk/Wv,
  attention for its 8 heads,
  partial out-proj y_g = outT_g^T @ Wo[rows g] (Wo row-sharded).
The two partials per batch are summed on the host (y = y_0 + y_1 + bo),
so no duplicated K/V compute and no device collective.

Per-core kernel layout (all matmuls contract over the partition dim):
  QT/KT [128, 4, 2048]   = W^T x^T   (partition block p = head pair p)
  Vg    [128, 16, 4*192] = V in per-pair [V_even |ones| V_odd] slots
                           (augmented V -> softmax denominator for free)
  scores: both heads of a pair packed into one [128,1024] PSUM tile via
          PE row-group tiling (K=64 matmuls at partition base 0 and 64
          run concurrently in separate row groups).
  exp:    split between ScalarE (exact activation, even jt) and VectorE
          (1-op Schraudolph fast-exp: round(a*s+b) as int16, bitcast
          bf16 ~= exp(s), odd jt) so neither engine bottlenecks.
  attnV:  [V_h|1]^T attnT accumulated over key tiles -> row block 64:128
          (or 0:64) = softmax denominator; normalize with DVE
          reciprocal + multiply.
  y_g [2048,1024] f32    = outT^T Wo_g  (host adds partials + bias)
"""

import os
import sys

for _p in ("/opt/trn_rl_repo", "/root/.axon_site/_ro/trn_rl_repo"):
    if os.path.isdir(_p) and _p not in sys.path:
        sys.path.append(_p)

import numpy as np
import ml_dtypes

import concourse.bacc as bacc
import concourse.mybir as mybir
import concourse.tile as tile
from concourse.bass_utils import run_bass_kernel_spmd
from contextlib import ExitStack

P = 128
DIM = 1024
HEADS = 16
DH = 64
NSEQ = 2048
GI = 512  # inner dims per core (8 heads)
KD = DIM // P  # 8 contraction tiles for projections
NPAIR = GI // P  # 4 head pairs per core
SCALE = 1.0 / DH**0.5
N_CORES = 8
NJT = NSEQ // P  # 16 key tiles
NQT = NSEQ // 512  # 4 query tiles of 512

CD = mybir.dt.bfloat16
NP_CD = ml_dtypes.bfloat16
F32 = mybir.dt.float32
I16 = mybir.dt.int16
Exp = mybir.ActivationFunctionType.Exp
MULT = mybir.AluOpType.mult
ADD = mybir.AluOpType.add

# Schraudolph fast-exp constants (bf16 = int16 bit pattern):
#   round(s * 128/ln2 + (16256 - c)) viewed as bf16 ~= exp(s)
EXP_A = 128.0 / float(np.log(2.0))
EXP_B = 16256.0 - 4.0

_CACHE = {}


def build_nc(unroll=1, fast_jt_mod=2, phases="ACD"):
    nc = bacc.Bacc(
        "TRN2", target_bir_lowering=False, debug=False, num_devices=N_CORES
    )

    xt_e = nc.dram_tensor("xt", [DIM, NSEQ], CD, kind="ExternalInput")
    wq_e = nc.dram_tensor("wq", [DIM, GI], CD, kind="ExternalInput")
    wk_e = nc.dram_tensor("wk", [DIM, GI], CD, kind="ExternalInput")
    wv_e = nc.dram_tensor("wv", [DIM, GI], CD, kind="ExternalInput")
    wo_e = nc.dram_tensor("wo", [GI, DIM], CD, kind="ExternalInput")
    y_e = nc.dram_tensor("y", [NSEQ, DIM], F32, kind="ExternalOutput")

    # DRAM views with the contraction dim on partitions
    xt_r = xt_e.ap().rearrange("(k p) n -> p k n", p=P)
    wq_r = wq_e.ap().rearrange("(k p) n -> p k n", p=P)
    wk_r = wk_e.ap().rearrange("(k p) n -> p k n", p=P)
    wv_r = wv_e.ap().rearrange("(k p) n -> p k n", p=P)
    wo_r = wo_e.ap().rearrange("(k p) n -> p k n", p=P)
    y_r = y_e.ap()

    with tile.TileContext(nc) as tc, ExitStack() as top:
        x_p = top.enter_context(tc.tile_pool(name="x_p", bufs=2))
        w_p = top.enter_context(tc.tile_pool(name="w_p", bufs=1))
        qt_p = top.enter_context(tc.tile_pool(name="qt_p", bufs=1))
        kt_p = top.enter_context(tc.tile_pool(name="kt_p", bufs=1))
        vg_p = top.enter_context(tc.tile_pool(name="vg_p", bufs=1))
        ot_p = top.enter_context(tc.tile_pool(name="ot_p", bufs=1))
        at_p = top.enter_context(tc.tile_pool(name="at_p", bufs=6))
        rb_p = top.enter_context(tc.tile_pool(name="rb_p", bufs=2))
        y_p = top.enter_context(tc.tile_pool(name="y_p", bufs=2))

        wq_sb = w_p.tile([P, KD, GI], CD)
        wk_sb = w_p.tile([P, KD, GI], CD)
        wv_sb = w_p.tile([P, KD, GI], CD)
        wo_sb = w_p.tile([P, NPAIR, DIM], CD)

        QT = qt_p.tile([P, NPAIR, NSEQ], CD)
        KT = kt_p.tile([P, NPAIR, NSEQ], CD)
        Vg = vg_p.tile([P, NJT, NPAIR * (4 * DH)], CD)
        outT = ot_p.tile([P, NPAIR, NSEQ], CD)

        # per pair: [ones 64 | V_even 64 | ones 64 | V_odd 64] — each head's
        # attnV lhsT is [ones|V_h], so the denominator rows land at PSUM
        # partition base 0 for BOTH heads (reciprocal_approx_fast needs
        # base 0; saves the even-head den copy)
        vg5 = Vg.rearrange("p t (pr two c) -> p t pr two c", two=2, c=2 * DH)
        nc.vector.memset(vg5[:, :, :, :, 0:DH], 1.0)

        def emit_body():
            # ---- phase A: projections ----
            # xt double-buffered: the next rep's reload overlaps this rep's
            # attention phase instead of stalling its phase A.
            xt_sb = x_p.tile([P, KD, NSEQ], CD, name="xt_sb", tag="xt_sb")
            for k in range(KD):
                nc.sync.dma_start(out=xt_sb[:, k, :], in_=xt_r[:, k, :])
                nc.sync.dma_start(out=wq_sb[:, k, :], in_=wq_r[:, k, :])
            for k in range(KD):
                nc.sync.dma_start(out=wk_sb[:, k, :], in_=wk_r[:, k, :])
            for k in range(KD):
                nc.sync.dma_start(out=wv_sb[:, k, :], in_=wv_r[:, k, :])
            for k in range(NPAIR):
                nc.sync.dma_start(out=wo_sb[:, k, :], in_=wo_r[:, k, :])

            with ExitStack() as es_a:
                psa_p = es_a.enter_context(
                    tc.tile_pool(name="psa_p", bufs=8, space="PSUM")
                )
                # Q and K -> [128, pair, seq] (ScalarE copies).
                # k-outer over 4 seq-tiles: each stationary weight tile
                # loads once per 4 matmuls so LDWEIGHTS stays hidden.
                for dst, w_sb in ((QT, wq_sb), (KT, wk_sb)):
                    for p in range(NPAIR):
                        pss = [
                            psa_p.tile([P, 512], F32, name="psa", tag="psa")
                            for _ in range(NQT)
                        ]
                        for k in range(KD):
                            for n in range(NQT):
                                nc.tensor.matmul(
                                    pss[n][:],
                                    w_sb[:, k, p * P : (p + 1) * P],
                                    xt_sb[:, k, n * 512 : (n + 1) * 512],
                                    start=(k == 0),
                                    stop=(k == KD - 1),
                                )
                        for n in range(NQT):
                            nc.scalar.copy(
                                dst[:, p, n * 512 : (n + 1) * 512], pss[n][:]
                            )
                # V -> Vg interleaved even/odd around the ones block (DVE)
                for t in range(NJT):
                    ps = psa_p.tile([P, 512], F32, name="psa", tag="psa")
                    for k in range(KD):
                        nc.tensor.matmul(
                            ps[:],
                            xt_sb[:, k, t * P : (t + 1) * P],
                            wv_sb[:, k, :],
                            start=(k == 0),
                            stop=(k == KD - 1),
                        )
                    ps_h = ps.rearrange("p (pr two c) -> p pr two c", two=2, c=DH)
                    nc.vector.tensor_copy(vg5[:, t, :, :, DH : 2 * DH], ps_h[:])

            if "C" not in phases:
                # minimal out write so partial-phase variants stay timeable
                yp = y_p.tile([P, DIM], F32, name="y", tag="y")
                nc.vector.tensor_copy(yp[:], QT[:, 0, 0:DIM])
                nc.sync.dma_start(out=y_r[0:P, :], in_=yp[:])
                return

            # ---- phase C (+ D interleaved per query tile) ----
            # exp lanes: ScalarE gets SC_JTS (+jt 13 on odd blocks), DVE the
            # rest via Schraudolph. D for query tile qt runs right after its
            # 4 pairs complete, spreading out-proj work and y DMAs.
            with ExitStack() as es_c:
                psc_p = es_c.enter_context(
                    tc.tile_pool(name="psc_p", bufs=3, space="PSUM")
                )
                po_p = es_c.enter_context(
                    tc.tile_pool(name="po_p", bufs=2, space="PSUM")
                )
                sc_base = frozenset((0, 3, 6, 9, 12, 15))
                for qt in range(NQT):
                    q0 = qt * 512
                    for pair in range(NPAIR):
                        vg_lo = pair * (4 * DH)
                        sc_jts = sc_base | (
                            frozenset((13,)) if (qt + pair) % 2 else frozenset()
                        )
                        po_e = po_p.tile([P, 512], F32, name="po", tag="po")
                        po_o = po_p.tile([P, 512], F32, name="po", tag="po")

                        def emit_scores(jt):
                            # both heads packed: even -> rows 0-63 tile,
                            # odd -> rows 64-127 tile (concurrent row groups)
                            ps = psc_p.tile([P, 1024], F32, name="psc", tag="psc")
                            nc.tensor.matmul(
                                ps[:, 0:512],
                                KT[0:DH, pair, jt * P : (jt + 1) * P],
                                QT[0:DH, pair, q0 : q0 + 512],
                                start=True,
                                stop=True,
                            )
                            nc.tensor.matmul(
                                ps[:, 512:1024],
                                KT[DH:P, pair, jt * P : (jt + 1) * P],
                                QT[DH:P, pair, q0 : q0 + 512],
                                start=True,
                                stop=True,
                            )
                            at = at_p.tile([P, 1024], CD, name="at", tag="at")
                            if jt in sc_jts:
                                nc.scalar.activation(at[:], ps[:], Exp)
                            else:
                                nc.vector.tensor_scalar(
                                    at[:].bitcast(I16), ps[:], EXP_A, EXP_B,
                                    MULT, ADD,
                                )
                            return at

                        def emit_attnv(jt, at):
                            nc.tensor.matmul(
                                po_e[:],
                                Vg[:, jt, vg_lo : vg_lo + 2 * DH],
                                at[:, 0:512],
                                start=(jt == 0),
                                stop=(jt == NJT - 1),
                            )
                            nc.tensor.matmul(
                                po_o[:],
                                Vg[:, jt, vg_lo + 2 * DH : vg_lo + 4 * DH],
                                at[:, 512:1024],
                                start=(jt == 0),
                                stop=(jt == NJT - 1),
                            )

                        # software pipeline depth 2: scores(jt+2) issues
                        # before attnv(jt) so exp latency stays hidden
                        ats = [emit_scores(0), emit_scores(1)]
                        for jt in range(NJT):
                            if jt + 2 < NJT:
                                ats.append(emit_scores(jt + 2))
                            emit_attnv(jt, ats[jt])

                        # normalize: both heads have rows [den 0:64 | out 64:128]
                        rbc_e = rb_p.tile([DH, 512], F32, name="rbc", tag="rbc")
                        nc.vector.reciprocal_approx_fast(rbc_e[:], po_e[0:DH, :])
                        nc.vector.tensor_tensor(
                            outT[0:DH, pair, q0 : q0 + 512],
                            po_e[DH:P, :],
                            rbc_e[:],
                            MULT,
                        )
                        rbc_o = rb_p.tile([DH, 512], F32, name="rbc", tag="rbc")
                        nc.vector.reciprocal_approx_fast(rbc_o[:], po_o[0:DH, :])
                        nc.vector.tensor_tensor(
                            outT[DH:P, pair, q0 : q0 + 512],
                            po_o[DH:P, :],
                            rbc_o[:],
                            MULT,
                        )

                    if "D" not in phases:
                        continue
                    # ---- D(qt): y rows for this query tile ----
                    for t in range(qt * 4, qt * 4 + 4):
                        ps = psc_p.tile([P, DIM], F32, name="psc", tag="psc")
                        for k in range(NPAIR):
                            for n in range(2):
                                nc.tensor.matmul(
                                    ps[:, n * 512 : (n + 1) * 512],
                                    outT[:, k, t * P : (t + 1) * P],
                                    wo_sb[:, k, n * 512 : (n + 1) * 512],
                                    start=(k == 0),
                                    stop=(k == NPAIR - 1),
                                )
                        y = y_p.tile([P, DIM], F32, name="y", tag="y")
                        nc.scalar.copy(y[:], ps[:])
                        nc.sync.dma_start(out=y_r[t * P : (t + 1) * P, :], in_=y[:])

            if "D" not in phases:
                yp = y_p.tile([P, DIM], F32, name="y", tag="y")
                nc.vector.tensor_copy(yp[:], outT[:, 0, 0:DIM])
                nc.sync.dma_start(out=y_r[0:P, :], in_=yp[:])

        for _rep in range(unroll):
            emit_body()

    nc.compile()
    return nc


def make_in_maps(x, Wq, Wkv, Wo, bo):
    x = np.asarray(x, dtype=np.float32)
    wq_s = (np.asarray(Wq, dtype=np.float32) * SCALE).astype(NP_CD)
    wk = np.asarray(Wkv, np.float32)[:, :DIM].astype(NP_CD)
    wv = np.asarray(Wkv, np.float32)[:, DIM:].astype(NP_CD)
    wo = np.asarray(Wo, dtype=np.float32).astype(NP_CD)

    xts = [np.ascontiguousarray(x[b].T).astype(NP_CD) for b in range(4)]
    in_maps = []
    for core in range(N_CORES):
        b, g = core // 2, core % 2
        lo, hi = g * GI, (g + 1) * GI
        in_maps.append(
            {
                "xt": xts[b],
                "wq": np.ascontiguousarray(wq_s[:, lo:hi]),
                "wk": np.ascontiguousarray(wk[:, lo:hi]),
                "wv": np.ascontiguousarray(wv[:, lo:hi]),
                "wo": np.ascontiguousarray(wo[lo:hi, :]),
            }
        )
    return in_maps


def kernel(x, Wq, Wkv, Wo, bo):
    if "nc" not in _CACHE:
        _CACHE["nc"] = build_nc()
    nc = _CACHE["nc"]
    in_maps = make_in_maps(x, Wq, Wkv, Wo, bo)
    res = run_bass_kernel_spmd(nc, in_maps, core_ids=list(range(N_CORES)))
    bo_f = np.asarray(bo, dtype=np.float32).reshape(1, DIM)
    out = np.empty((4, NSEQ, DIM), dtype=np.float32)
    for b in range(4):
        out[b] = res.results[2 * b]["y"] + res.results[2 * b + 1]["y"] + bo_f
    return out


if __name__ == "__main__":
    rng = np.random.default_rng(0)
    x = rng.standard_normal((4, NSEQ, DIM), dtype=np.float32)
    Wq = rng.standard_normal((DIM, DIM), dtype=np.float32) / 32
    Wkv = rng.standard_normal((DIM, 2 * DIM), dtype=np.float32) / 32
    Wo = rng.standard_normal((DIM, DIM), dtype=np.float32) / 32
    bo = rng.standard_normal((DIM,), dtype=np.float32) * 0.01
    out = kernel(x=x, Wq=Wq, Wkv=Wkv, Wo=Wo, bo=bo)

    # numpy reference
    def ref(x, Wq, Wkv, Wo, bo):
        b, n, _ = x.shape
        q = x @ Wq
        kv = x @ Wkv
        k, v = kv[:, :, :DIM], kv[:, :, DIM:]
        q = q.reshape(b, n, HEADS, DH).transpose(0, 2, 1, 3)
        k = k.reshape(b, n, HEADS, DH).transpose(0, 2, 1, 3)
        v = v.reshape(b, n, HEADS, DH).transpose(0, 2, 1, 3)
        s = np.einsum("bhid,bhjd->bhij", q, k) * SCALE
        s = s - s.max(-1, keepdims=True)
        a = np.exp(s)
        a = a / a.sum(-1, keepdims=True)
        o = np.einsum("bhij,bhjd->bhid", a, v)
        o = o.transpose(0, 2, 1, 3).reshape(b, n, HEADS * DH)
        return o @ Wo + bo

    exp = ref(x, Wq, Wkv, Wo, bo)
    err = np.linalg.norm(out - exp) / np.linalg.norm(exp)
    print("out", out.shape, out.dtype, "rel err:", err)
